# revision 1
# baseline (speedup 1.0000x reference)
"""GCN encoder (GIN conv -> 2x GCN conv) on 8 Trainium2 NeuronCores.

Strategy (dst-sharded, graph-parallel):
- Nodes are sharded by dst across 8 cores (12500 each). Each core owns the
  segment-sums and all dense math for its nodes; weights are replicated.
- Within a core, nodes are sorted by in-degree and grouped into 98 blocks of
  128; each block is padded to its max degree D_b, giving a dense
  [D_b, 128, 64] "slot" layout where tile s holds the s-th in-edge message of
  each of the 128 nodes. The segment-sum is then a chain of D_b TensorE
  matmuls accumulating into PSUM (lhsT = message tile, rhs = identity), which
  yields the aggregate directly in feature-major layout for the following
  linear layers.
- Per-edge message rows (x[src] for the GIN pass, the dinv-scaled
  concatenated GCN projections m[src] for the fused mu/logvar pass) are
  materialized into the slot layout on the host as part of input sharding;
  the device consumes them as dense streams at full DMA bandwidth.
- GCN normalization dinv[src]*dinv[dst] is factored: the table rows are
  pre-scaled by dinv[src] on device (launch A epilogue), and the dst factor
  is applied after the segment-sum (launch C epilogue), so no per-edge
  normalization gather is needed:
      out_i = dinv_i * (sum_{j->i} m_j + m_i) + b,   m_j = dinv_j * (h W)_j

Two SPMD launches:
  A: slots1 (x[src] rows) -> agg -> h = relu((x+agg) gin_W + gin_b)
     -> m = dinv * (h [mu_W|lv_W])          (per-core slice, feature-major)
  C: slots2 (m[src] rows) -> segment-sum -> epilogue -> [mu|logvar]
Host between launches: assemble the m table from the 8 slices and gather it
into the pass-2 slot layout (same index structure as pass 1).
"""

import numpy as np
import ml_dtypes

BF16 = ml_dtypes.bfloat16

N = 100000
E = 1600000
CIN = 64
HID = 64
COUT = 32
NCORES = 8
NPC = N // NCORES            # 12500 real nodes per core
BLK = 128
NBLK = 100                   # blocks per core (multiple of SB=4)
SB = 4                       # blocks per supertile (shares one PSUM bank)
NPCP = NBLK * BLK            # 12800 padded positions per core

_cache = {}


def _build_programs(d_sched):
    import concourse.bass as bass
    import concourse.bacc as bacc
    import concourse.mybir as mybir
    import concourse.tile as tile
    from concourse.masks import make_identity

    t1 = int(np.sum(d_sched))
    tile_off = np.concatenate([[0], np.cumsum(d_sched)]).astype(int)
    gdmax = max(int(tile_off[g + SB] - tile_off[g])
                for g in range(0, NBLK, SB))

    def build(which):
        nc = bacc.Bacc("TRN2", target_bir_lowering=False, debug=False,
                       enable_asserts=False, num_devices=NCORES)
        slots = nc.dram_tensor("slots", [BLK, t1, 64], mybir.dt.bfloat16,
                               kind="ExternalInput").ap()
        selfT = nc.dram_tensor("selfT", [64, NPCP], mybir.dt.float32,
                               kind="ExternalInput").ap()
        dinvT = nc.dram_tensor("dinvT", [64, NPCP], mybir.dt.float32,
                               kind="ExternalInput").ap()
        if which == "A":
            ginW = nc.dram_tensor("ginW", [64, 64], mybir.dt.bfloat16,
                                  kind="ExternalInput").ap()
            ginb = nc.dram_tensor("ginb", [64, 1], mybir.dt.float32,
                                  kind="ExternalInput").ap()
            wcat = nc.dram_tensor("wcat", [64, 64], mybir.dt.bfloat16,
                                  kind="ExternalInput").ap()
            outT = nc.dram_tensor("outT", [64, NPCP], mybir.dt.bfloat16,
                                  kind="ExternalOutput").ap()
        else:
            bias = nc.dram_tensor("bias", [64, 1], mybir.dt.float32,
                                  kind="ExternalInput").ap()
            outT = nc.dram_tensor("outT", [64, NPCP], mybir.dt.float32,
                                  kind="ExternalOutput").ap()

        with tile.TileContext(nc) as tc:
            with (tc.tile_pool(name="const", bufs=1) as cpool,
                  tc.tile_pool(name="blkin", bufs=3) as bpool,
                  tc.tile_pool(name="work", bufs=3) as wpool,
                  tc.tile_pool(name="ps", bufs=(3 if which == "A" else 6),
                               space="PSUM") as ppool,
                  tc.tile_pool(name="ps2", bufs=2, space="PSUM") as p2pool):
                ident = cpool.tile([BLK, BLK], mybir.dt.bfloat16)
                make_identity(nc, ident[:])
                CHB = 20 if NBLK % 20 == 0 else SB  # blocks per const chunk
                CHW = CHB * BLK
                selfT_sb = []
                dinvT_sb = []
                for k in range(NBLK // CHB):
                    st = cpool.tile([64, CHW], mybir.dt.float32,
                                    tag=f"selfT{k}")
                    nc.scalar.dma_start(out=st[:],
                                        in_=selfT[:, k * CHW:(k + 1) * CHW])
                    selfT_sb.append(st)
                    dt_ = cpool.tile([64, CHW], mybir.dt.float32,
                                     tag=f"dinvT{k}")
                    nc.scalar.dma_start(out=dt_[:],
                                        in_=dinvT[:, k * CHW:(k + 1) * CHW])
                    dinvT_sb.append(dt_)
                if which == "A":
                    ginW_sb = cpool.tile([64, 64], mybir.dt.bfloat16)
                    nc.scalar.dma_start(out=ginW_sb[:], in_=ginW[:])
                    ginb_sb = cpool.tile([64, 1], mybir.dt.float32)
                    nc.scalar.dma_start(out=ginb_sb[:], in_=ginb[:])
                    wcat_sb = cpool.tile([64, 64], mybir.dt.bfloat16)
                    nc.scalar.dma_start(out=wcat_sb[:], in_=wcat[:])
                else:
                    bias_sb = cpool.tile([64, 1], mybir.dt.float32)
                    nc.scalar.dma_start(out=bias_sb[:], in_=bias[:])

                W = SB * BLK                 # supertile width (512)
                for g in range(NBLK // SB):
                    b0 = g * SB
                    gsl = slice(b0 * BLK, (b0 + SB) * BLK)
                    ck = b0 // CHB
                    csl = slice((b0 % CHB) * BLK, (b0 % CHB) * BLK + W)
                    selfT_g = selfT_sb[ck]
                    dinvT_g = dinvT_sb[ck]
                    gt0 = int(tile_off[b0])
                    gtn = int(tile_off[b0 + SB] - gt0)
                    # one DMA brings all SB blocks' slot tiles
                    blkt = bpool.tile([BLK, gdmax * 64], mybir.dt.bfloat16,
                                      tag="blk")
                    nc.sync.dma_start(
                        out=blkt[:, :gtn * 64],
                        in_=slots[:, gt0:gt0 + gtn, :],
                    )
                    # one PSUM bank holds all SB blocks' [128,128] psums
                    ps = ppool.tile([BLK, W], mybir.dt.float32, space="PSUM")
                    for j in range(SB):
                        b = b0 + j
                        db = int(d_sched[b])
                        o = int(tile_off[b] - gt0)
                        for s in range(db // 2):
                            nc.tensor.matmul(
                                out=ps[:, j * BLK:(j + 1) * BLK],
                                lhsT=blkt[:, (o + s * 2) * 64:
                                          (o + s * 2 + 2) * 64],
                                rhs=ident[:],
                                start=(s == 0),
                                stop=(s == db // 2 - 1),
                            )
                    # (top + selfT) + bottom, one wide op each
                    t0 = wpool.tile([64, W], mybir.dt.float32, tag="t0")
                    nc.vector.tensor_add(out=t0[:], in0=ps[0:64, :],
                                         in1=selfT_g[:, csl])
                    if which == "A":
                        xin = wpool.tile([64, W], mybir.dt.bfloat16,
                                         tag="xin")
                        nc.vector.tensor_add(
                            out=xin[:], in0=t0[:], in1=ps[64:BLK, :])
                        ps2 = p2pool.tile([64, W], mybir.dt.float32,
                                          space="PSUM")
                        nc.tensor.matmul(out=ps2[:], lhsT=ginW_sb[:],
                                         rhs=xin[:], start=True, stop=True)
                        hT = wpool.tile([64, W], mybir.dt.bfloat16,
                                        tag="hT")
                        nc.scalar.activation(
                            hT[:], ps2[:],
                            mybir.ActivationFunctionType.Relu,
                            bias=ginb_sb[:], scale=1.0)
                        ps3 = p2pool.tile([64, W], mybir.dt.float32,
                                          space="PSUM")
                        nc.tensor.matmul(out=ps3[:], lhsT=wcat_sb[:],
                                         rhs=hT[:], start=True, stop=True)
                        ot = wpool.tile([64, W], mybir.dt.bfloat16,
                                        tag="ot")
                        nc.vector.tensor_mul(
                            out=ot[:], in0=ps3[:], in1=dinvT_g[:, csl])
                        nc.scalar.dma_start(out=outT[:, gsl], in_=ot[:])
                    else:
                        t1w = wpool.tile([64, W], mybir.dt.float32,
                                         tag="t1")
                        nc.vector.tensor_add(
                            out=t1w[:], in0=t0[:], in1=ps[64:BLK, :])
                        ot = wpool.tile([64, W], mybir.dt.float32,
                                        tag="ot")
                        nc.vector.tensor_mul(
                            out=ot[:], in0=t1w[:], in1=dinvT_g[:, csl])
                        # mu rows: relu(ot + mu_b) in place via ACT
                        nc.scalar.activation(
                            ot[0:COUT, :], ot[0:COUT, :],
                            mybir.ActivationFunctionType.Relu,
                            bias=bias_sb[0:COUT, :], scale=1.0)
                        nc.scalar.dma_start(out=outT[:, gsl], in_=ot[:])
        nc.compile()
        from concourse.bass_interp import get_hw_module
        nc.m = get_hw_module(nc.m)
        return nc

    return build("A"), build("C")


def _prep(edge_index):
    """Shard/sort/pad the graph; returns per-core index structures."""
    src = np.asarray(edge_index[0], dtype=np.int64)
    dst = np.asarray(edge_index[1], dtype=np.int64)
    deg_in = np.bincount(dst, minlength=N)
    dinv = (1.0 / np.sqrt(deg_in + 1.0)).astype(np.float32)

    cores = []
    d_sched_per_core = np.zeros((NCORES, NBLK), dtype=np.int64)
    for c in range(NCORES):
        lo, hi = c * NPC, (c + 1) * NPC
        m = (dst >= lo) & (dst < hi)
        s_c = src[m]
        d_c = (dst[m] - lo).astype(np.int64)
        deg_c = np.bincount(d_c, minlength=NPC)
        order = np.argsort(deg_c, kind="stable")      # position -> local node
        pos = np.empty(NPC, dtype=np.int64)
        pos[order] = np.arange(NPC)                   # local node -> position
        posdeg = np.zeros(NPCP, dtype=np.int64)
        posdeg[:NPC] = deg_c[order]
        d_sched_per_core[c] = posdeg.reshape(NBLK, BLK).max(axis=1)
        cores.append((s_c, d_c, deg_c, order, pos, posdeg))

    d_sched = d_sched_per_core.max(axis=0)
    d_sched = np.maximum(d_sched, 1)
    d_sched = ((d_sched + 1) // 2) * 2        # even: paired matmuls
    t1 = int(d_sched.sum())
    tile_off = np.concatenate([[0], np.cumsum(d_sched)]).astype(np.int64)

    srcidx = np.full((NCORES, t1, BLK), -1, dtype=np.int64)
    pos_of_global = np.empty(N, dtype=np.int64)
    for c in range(NCORES):
        s_c, d_c, deg_c, order, pos, posdeg = cores[c]
        pos_of_global[c * NPC + order] = c * NPCP + np.arange(NPC)
        key = pos[d_c]
        eord = np.argsort(key, kind="stable")
        spos = key[eord]
        start_of_pos = np.zeros(NPCP, dtype=np.int64)
        np.cumsum(posdeg[:-1], out=start_of_pos[1:])
        r = np.arange(len(spos)) - start_of_pos[spos]
        t = tile_off[spos // BLK] + r
        srcidx[c, t, spos % BLK] = s_c[eord]
    return d_sched, t1, srcidx, pos_of_global, dinv, cores


TRACE = False
last_exec_ns = []


def _run(nc, in_maps):
    from concourse import bass_utils
    res = bass_utils.run_bass_kernel_spmd(nc, in_maps,
                                          core_ids=list(range(NCORES)),
                                          trace=TRACE)
    if TRACE:
        last_exec_ns.append(res.exec_time_ns)
    return res.results


def kernel(x, edge_index, gin_W, gin_b, mu_W, mu_b, lv_W, lv_b):
    x = np.asarray(x, dtype=np.float32)
    gin_W = np.asarray(gin_W, dtype=np.float32)
    gin_b = np.asarray(gin_b, dtype=np.float32)
    wcat = np.concatenate([np.asarray(mu_W, np.float32),
                           np.asarray(lv_W, np.float32)], axis=1)
    bias_cat = np.concatenate([np.asarray(mu_b, np.float32),
                               np.asarray(lv_b, np.float32)])

    d_sched, t1, srcidx, pos_of_global, dinv, cores = _prep(edge_index)

    key = ("prog", t1, tuple(int(v) for v in d_sched))
    if key not in _cache:
        _cache[key] = _build_programs(d_sched)
    nc_A, nc_C = _cache[key]

    # ---- launch A inputs ----
    x_pad = np.zeros((N + 1, 64), dtype=BF16)
    x_pad[:N] = x.astype(BF16)
    gather1 = np.where(srcidx >= 0, srcidx, N)

    in_maps_A = []
    for c in range(NCORES):
        _, _, _, order, _, _ = cores[c]
        xT = np.zeros((64, NPCP), dtype=np.float32)
        xT[:, :NPC] = x[c * NPC + order].T.astype(BF16).astype(np.float32)
        dT = np.ones((NPCP,), dtype=np.float32)
        dT[:NPC] = dinv[c * NPC + order]
        dinvT = np.broadcast_to(dT, (64, NPCP)).copy()
        in_maps_A.append({
            "slots": np.ascontiguousarray(
                x_pad[gather1[c]].transpose(1, 0, 2)),
            "selfT": xT,
            "dinvT": dinvT,
            "ginW": gin_W.astype(BF16),
            "ginb": gin_b.reshape(64, 1),
            "wcat": wcat.astype(BF16),
        })
    res_A = _run(nc_A, in_maps_A)

    # ---- assemble m table, build launch C inputs ----
    m_pos = np.zeros((NCORES * NPCP + 1, 64), dtype=BF16)
    for c in range(NCORES):
        m_pos[c * NPCP:(c + 1) * NPCP] = res_A[c]["outT"].T
    gather2 = np.where(srcidx >= 0, pos_of_global[srcidx],
                       NCORES * NPCP)

    in_maps_C = []
    for c in range(NCORES):
        in_maps_C.append({
            "slots": np.ascontiguousarray(
                m_pos[gather2[c]].transpose(1, 0, 2)),
            "selfT": m_pos[c * NPCP:(c + 1) * NPCP].T.astype(np.float32),
            "dinvT": in_maps_A[c]["dinvT"],
            "bias": np.concatenate([bias_cat[:COUT],
                                    np.zeros(COUT, np.float32)]
                                   ).reshape(64, 1),
        })
    res_C = _run(nc_C, in_maps_C)

    # ---- unshard ----
    out = np.empty((N, 64), dtype=np.float32)
    for c in range(NCORES):
        _, _, _, order, _, _ = cores[c]
        out[c * NPC + order] = res_C[c]["outT"][:, :NPC].T
    return out[:, :COUT], out[:, COUT:] + bias_cat[COUT:]



# revision 6
# speedup vs baseline: 1.7283x; 1.7283x over previous
"""GCN encoder (GIN conv -> 2x GCN conv) on 8 Trainium2 NeuronCores.

Strategy (dst-sharded, graph-parallel, fp8-e3m4 message streams):
- Nodes sharded by dst across 8 cores (12500 each); each core owns the
  segment-sums and dense math for its nodes; weights replicated.
- Self-loops ride the edge stream: a synthetic (i, i) edge is appended per
  node, so the device only ever sums slot rows (no separate self term).
- Within a core, nodes are sorted by in-degree and grouped into 100 blocks
  of 128; each block padded to its max degree D_b, giving a dense
  [D_b, 128, 64] slot layout. Aggregation is a chain of TensorE matmuls
  (lhsT = slot pair, rhs = identity) accumulating the transposed sum in
  PSUM, feature-major for the following linear layers.
- Slot rows are stored as float8 e3m4 with a single global scale chosen so
  absmax ~ 15 (e3m4 max 15.5). Quantization is absolute-error-optimal-ish
  for the rel-err metric; the scale is undone for free:
    launch A: gin_W is pre-scaled by s1 on host,
    launch C: the ACT epilogue applies scale=s2 via its scale operand.
- GCN normalization dinv_i*dinv_j is folded into the host gather that
  builds launch C's slot rows (host already touches every byte there), so
  no dinv stream or multiply exists on device.

Two SPMD launches:
  A: slots1 (x[src]/s1 rows, e3m4) -> transpose-sum -> xin(bf16)
     -> h = relu(xin @ (s1*gin_W) + gin_b) -> p = h @ [mu_W|lv_W] (bf16)
  C: slots2 ((dinv_i dinv_j p_j)/s2 rows, e3m4) -> transpose-sum
     -> out = act(s2*sum + bias)  (relu for mu rows, identity for logvar)
Host between launches: gather p into the pass-2 slot layout with the dinv
product and 1/s2 scaling applied during the gather.
"""

import numpy as np
import ml_dtypes

BF16 = ml_dtypes.bfloat16
E3M4 = ml_dtypes.float8_e3m4

N = 100000
E = 1600000
CIN = 64
HID = 64
COUT = 32
NCORES = 8
NPC = N // NCORES            # 12500 real nodes per core
BLK = 128
NBLK = 100                   # blocks per core
SB = 4                       # blocks per supertile (shares one PSUM bank)
GRPB = 8                     # blocks per slot DMA (2 supertiles)
NPCP = NBLK * BLK            # 12800 padded positions per core
AMAX = 15.0                  # e3m4 target absmax (max normal 15.5)

_cache = {}


def _build_programs(d_sched):
    import concourse.bass as bass
    import concourse.bacc as bacc
    import concourse.mybir as mybir
    import concourse.tile as tile

    t1 = int(np.sum(d_sched))
    tile_off = np.concatenate([[0], np.cumsum(d_sched)]).astype(int)
    gd8 = max(int(tile_off[min(g + GRPB, NBLK)] - tile_off[g])
              for g in range(0, NBLK, GRPB))

    def build(which):
        nc = bacc.Bacc("TRN2", target_bir_lowering=False, debug=False,
                       enable_asserts=False, num_devices=NCORES)
        slots = nc.dram_tensor("slots", [BLK, t1, 64], mybir.dt.float8e3,
                               kind="ExternalInput").ap()
        identD = nc.dram_tensor("identD", [BLK, BLK], mybir.dt.float8e3,
                                kind="ExternalInput").ap()
        if which == "A":
            ginW = nc.dram_tensor("ginW", [128, 64], mybir.dt.bfloat16,
                                  kind="ExternalInput").ap()
            ginb = nc.dram_tensor("ginb", [64, 1], mybir.dt.float32,
                                  kind="ExternalInput").ap()
            wcat = nc.dram_tensor("wcat", [64, 64], mybir.dt.bfloat16,
                                  kind="ExternalInput").ap()
        else:
            sdup = nc.dram_tensor("sdup", [128, 64], mybir.dt.bfloat16,
                                  kind="ExternalInput").ap()
            bias = nc.dram_tensor("bias", [64, 1], mybir.dt.float32,
                                  kind="ExternalInput").ap()
            scl = nc.dram_tensor("scl", [64, 1], mybir.dt.float32,
                                 kind="ExternalInput").ap()
        outT = nc.dram_tensor("outT", [64, NPCP], mybir.dt.bfloat16,
                              kind="ExternalOutput").ap()

        with tile.TileContext(nc) as tc:
            with (tc.tile_pool(name="const", bufs=1) as cpool,
                  tc.tile_pool(name="blkin", bufs=3) as bpool,
                  tc.tile_pool(name="work", bufs=4) as wpool,
                  tc.tile_pool(name="ps", bufs=(4 if which == "A" else 6),
                               space="PSUM") as ppool,
                  tc.tile_pool(name="ps2", bufs=2, space="PSUM") as p2pool):
                ident = cpool.tile([BLK, BLK], mybir.dt.float8e3)
                nc.scalar.dma_start(out=ident[:], in_=identD[:])
                if which == "A":
                    ginW_sb = cpool.tile([128, 64], mybir.dt.bfloat16)
                    nc.scalar.dma_start(out=ginW_sb[:], in_=ginW[:])
                    ginb_sb = cpool.tile([64, 1], mybir.dt.float32)
                    nc.scalar.dma_start(out=ginb_sb[:], in_=ginb[:])
                    wcat_sb = cpool.tile([64, 64], mybir.dt.bfloat16)
                    nc.scalar.dma_start(out=wcat_sb[:], in_=wcat[:])
                else:
                    sdup_sb = cpool.tile([128, 64], mybir.dt.bfloat16)
                    nc.scalar.dma_start(out=sdup_sb[:], in_=sdup[:])
                    bias_sb = cpool.tile([64, 1], mybir.dt.float32)
                    nc.scalar.dma_start(out=bias_sb[:], in_=bias[:])
                    scl_sb = cpool.tile([64, 1], mybir.dt.float32)
                    nc.scalar.dma_start(out=scl_sb[:], in_=scl[:])

                W = SB * BLK                 # supertile width (512)
                for g0 in range(0, NBLK, GRPB):
                    nb = min(GRPB, NBLK - g0)
                    gt0 = int(tile_off[g0])
                    gtn = int(tile_off[g0 + nb] - gt0)
                    blkt = bpool.tile([BLK, gd8 * 64], mybir.dt.float8e3,
                                      tag="blk")
                    nc.sync.dma_start(
                        out=blkt[:, :gtn * 64],
                        in_=slots[:, gt0:gt0 + gtn, :],
                    )
                    for si in range(nb // SB):
                        b0 = g0 + si * SB
                        gsl = slice(b0 * BLK, (b0 + SB) * BLK)
                        ps = ppool.tile([BLK, W], mybir.dt.float32,
                                        space="PSUM")
                        for j in range(SB):
                            b = b0 + j
                            db = int(d_sched[b])
                            o = int(tile_off[b] - gt0)
                            for s in range(db // 2):
                                nc.tensor.matmul(
                                    out=ps[:, j * BLK:(j + 1) * BLK],
                                    lhsT=blkt[:, (o + s * 2) * 64:
                                              (o + s * 2 + 2) * 64],
                                    rhs=ident[:],
                                    start=(s == 0),
                                    stop=(s == db // 2 - 1),
                                )
                        if which == "A":
                            xin = wpool.tile([BLK, W], mybir.dt.bfloat16,
                                             tag="xin")
                            nc.vector.tensor_scalar_mul(xin[:], ps[:], 1.0)
                            ps2 = p2pool.tile([64, W], mybir.dt.float32,
                                              space="PSUM")
                            nc.tensor.matmul(out=ps2[:], lhsT=ginW_sb[:],
                                             rhs=xin[:], start=True, stop=True)
                            hT = wpool.tile([64, W], mybir.dt.bfloat16,
                                            tag="hT")
                            nc.scalar.activation(
                                hT[:], ps2[:],
                                mybir.ActivationFunctionType.Relu,
                                bias=ginb_sb[:], scale=1.0)
                            ps3 = p2pool.tile([64, W], mybir.dt.float32,
                                              space="PSUM")
                            nc.tensor.matmul(out=ps3[:], lhsT=wcat_sb[:],
                                             rhs=hT[:], start=True, stop=True)
                            ot = wpool.tile([64, W], mybir.dt.bfloat16,
                                            tag="ot")
                            nc.vector.tensor_scalar_mul(ot[:], ps3[:], 1.0)
                            nc.scalar.dma_start(out=outT[:, gsl], in_=ot[:])
                        else:
                            xcp = wpool.tile([BLK, W], mybir.dt.bfloat16,
                                             tag="xcp")
                            nc.vector.tensor_scalar_mul(xcp[:], ps[:], 1.0)
                            psc = p2pool.tile([64, W], mybir.dt.float32,
                                              space="PSUM")
                            nc.tensor.matmul(out=psc[:], lhsT=sdup_sb[:],
                                             rhs=xcp[:], start=True, stop=True)
                            ot = wpool.tile([64, W], mybir.dt.bfloat16,
                                            tag="ot")
                            nc.scalar.activation(
                                ot[0:COUT, :], psc[0:COUT, :],
                                mybir.ActivationFunctionType.Relu,
                                bias=bias_sb[0:COUT, :],
                                scale=scl_sb[0:COUT, :])
                            nc.scalar.activation(
                                ot[COUT:64, :], psc[COUT:64, :],
                                mybir.ActivationFunctionType.Identity,
                                bias=bias_sb[COUT:64, :],
                                scale=scl_sb[COUT:64, :])
                            nc.scalar.dma_start(out=outT[:, gsl], in_=ot[:])
        nc.compile()
        from concourse.bass_interp import get_hw_module
        nc.m = get_hw_module(nc.m)
        return nc

    return build("A"), build("C")


class _null_ctx:
    def __enter__(self):
        return None

    def __exit__(self, *a):
        return False


def _prep(edge_index):
    """Shard/sort/pad the graph (self-loops appended as real edges)."""
    src0 = np.asarray(edge_index[0], dtype=np.int64)
    dst0 = np.asarray(edge_index[1], dtype=np.int64)
    deg_in = np.bincount(dst0, minlength=N)
    dinv = (1.0 / np.sqrt(deg_in + 1.0)).astype(np.float32)
    allN = np.arange(N, dtype=np.int64)
    src = np.concatenate([src0, allN])
    dst = np.concatenate([dst0, allN])

    cores = []
    d_sched_per_core = np.zeros((NCORES, NBLK), dtype=np.int64)
    for c in range(NCORES):
        lo, hi = c * NPC, (c + 1) * NPC
        m = (dst >= lo) & (dst < hi)
        s_c = src[m]
        d_c = (dst[m] - lo).astype(np.int64)
        deg_c = np.bincount(d_c, minlength=NPC)
        order = np.argsort(deg_c, kind="stable")      # position -> local node
        pos = np.empty(NPC, dtype=np.int64)
        pos[order] = np.arange(NPC)                   # local node -> position
        posdeg = np.zeros(NPCP, dtype=np.int64)
        posdeg[:NPC] = deg_c[order]
        d_sched_per_core[c] = posdeg.reshape(NBLK, BLK).max(axis=1)
        cores.append((s_c, d_c, order, pos, posdeg))

    d_sched = d_sched_per_core.max(axis=0)
    d_sched = np.maximum(d_sched, 2)
    d_sched = ((d_sched + 1) // 2) * 2        # even: paired matmuls
    t1 = int(d_sched.sum())
    tile_off = np.concatenate([[0], np.cumsum(d_sched)]).astype(np.int64)

    srcidx = np.full((NCORES, t1, BLK), -1, dtype=np.int64)
    coefsl = np.zeros((NCORES, t1, BLK), dtype=np.float32)
    pos_of_global = np.empty(N, dtype=np.int64)
    for c in range(NCORES):
        s_c, d_c, order, pos, posdeg = cores[c]
        pos_of_global[c * NPC + order] = c * NPCP + np.arange(NPC)
        key = pos[d_c]
        eord = np.argsort(key, kind="stable")
        spos = key[eord]
        start_of_pos = np.zeros(NPCP, dtype=np.int64)
        np.cumsum(posdeg[:-1], out=start_of_pos[1:])
        r = np.arange(len(spos)) - start_of_pos[spos]
        t = tile_off[spos // BLK] + r
        se = s_c[eord]
        de = d_c[eord] + c * NPC                      # global dst node
        srcidx[c, t, spos % BLK] = se
        # dinv_i * dinv_j for the edge landing in this slot (i = dst owner)
        coefsl[c, t, spos % BLK] = dinv[se] * dinv[de]
    return d_sched, t1, srcidx, coefsl, pos_of_global, dinv, cores


TRACE = False
last_exec_ns = []


def _run(nc, in_maps):
    from concourse import bass_utils
    res = bass_utils.run_bass_kernel_spmd(nc, in_maps,
                                          core_ids=list(range(NCORES)),
                                          trace=TRACE)
    if TRACE:
        last_exec_ns.append(res.exec_time_ns)
    return res.results


def kernel(x, edge_index, gin_W, gin_b, mu_W, mu_b, lv_W, lv_b):
    x = np.asarray(x, dtype=np.float32)
    gin_W = np.asarray(gin_W, dtype=np.float32)
    gin_b = np.asarray(gin_b, dtype=np.float32)
    wcat = np.concatenate([np.asarray(mu_W, np.float32),
                           np.asarray(lv_W, np.float32)], axis=1)
    bias_cat = np.concatenate([np.asarray(mu_b, np.float32),
                               np.asarray(lv_b, np.float32)])

    d_sched, t1, srcidx, coefsl, pos_of_global, dinv, cores = _prep(edge_index)

    key = ("prog", t1, tuple(int(v) for v in d_sched))
    if key not in _cache:
        _cache[key] = _build_programs(d_sched)
    nc_A, nc_C = _cache[key]

    identM = np.eye(BLK, dtype=np.float32).astype(E3M4)

    # ---- launch A inputs ----
    s1 = float(np.abs(x).max()) / AMAX
    xq = (x / s1).astype(E3M4)
    x_pad = np.zeros((N + 1, 64), dtype=E3M4)
    x_pad[:N] = xq
    gather1 = np.where(srcidx >= 0, srcidx, N)

    in_maps_A = []
    for c in range(NCORES):
        in_maps_A.append({
            "slots": np.ascontiguousarray(
                x_pad[gather1[c]].transpose(1, 0, 2)),
            "identD": identM,
            "ginW": np.vstack([s1 * gin_W, s1 * gin_W]).astype(BF16),
            "ginb": gin_b.reshape(64, 1),
            "wcat": wcat.astype(BF16),
        })
    res_A = _run(nc_A, in_maps_A)

    # ---- assemble p table, build launch C inputs ----
    p_pos = np.zeros((NCORES * NPCP + 1, 64), dtype=np.float32)
    for c in range(NCORES):
        p_pos[c * NPCP:(c + 1) * NPCP] = res_A[c]["outT"].T
    gather2 = np.where(srcidx >= 0, pos_of_global[srcidx],
                       NCORES * NPCP)

    rowmax = np.abs(p_pos).max(axis=1)
    s2 = 0.0
    for c in range(NCORES):
        s2 = max(s2, float((coefsl[c] * rowmax[gather2[c]]).max()))
    s2 /= AMAX

    in_maps_C = []
    for c in range(NCORES):
        vals = p_pos[gather2[c]] * (coefsl[c] / s2)[:, :, None]
        in_maps_C.append({
            "slots": np.ascontiguousarray(
                vals.astype(E3M4).transpose(1, 0, 2)),
            "identD": identM,
            "sdup": np.tile(np.eye(64, dtype=np.float32), (2, 1)).astype(BF16),
            "bias": bias_cat.reshape(64, 1).astype(np.float32),
            "scl": np.full((64, 1), s2, dtype=np.float32),
        })
    res_C = _run(nc_C, in_maps_C)

    # ---- unshard ----
    out = np.empty((N, 64), dtype=np.float32)
    for c in range(NCORES):
        _, _, order, _, _ = cores[c]
        out[c * NPC + order] = res_C[c]["outT"][:, :NPC].T
    return out[:, :COUT], out[:, COUT:]
